# revision 23
# baseline (speedup 1.0000x reference)
"""Trainium2 Bass kernel for nn_DeconvCNNLoss.

Computes  sum_{b,l} exp(s[b,l]/tau) / sum_v exp(dist[b,l,v]/tau)
with  dist = einsum('bel,ve->blv', embed_DE, embed_M)
and   s    = sum_e embed_EN * embed_DE.

Sharding: tensor-parallel over the vocab dim V across 8 cores.  Each core
receives embed_M's shard pre-transposed to [E, V/8] (layout choice made on
the host while sharding), the full embed_DE / embed_EN, and produces
partial exp-sum denominators for all B*L tokens plus the numerator dot
products.  The host sums the 8 partial denominators (the "all-reduce"),
applies exp to the numerator and does the final division + scalar sum.

Matmul operands are fed as bf16: trn2 f32 matmuls decompose into LOW/HIGH
PE passes (~4x the cycles measured); bf16 with f32 PSUM accumulation keeps
the final loss within ~1e-3 relative while running the PE at full rate.

Device work per core:
  - 512 bf16 matmuls [128e,128l]^T @ [128e,500v] accumulated over e in PSUM
  - fused exp+row-sum on the scalar engine (activation Exp with accum_out)
  - numerator: DVE elementwise EN*DE, partition-reduced with a ones-matmul
"""

import numpy as np

B, E, L, V = 4, 512, 512, 32000
NCORES = 8
VS = V // NCORES          # 4000 vocab rows per core
VBLK = 500                # vocab columns per matmul (one PSUM bank)
NVB = VS // VBLK          # 8 vocab blocks per core
NLB = L // 128            # 4 token blocks per batch entry
NTB = B * NLB             # 16 token blocks total
NKB = E // 128            # 4 contraction blocks
INV_TAU = 0.1

_CACHE = {}
LAST_RESULTS = None       # test.py reads exec_time_ns from here


def _build():
    from contextlib import ExitStack

    import concourse.bacc as bacc
    import concourse.mybir as mybir
    import concourse.tile as tile

    f32 = mybir.dt.float32
    bf16 = mybir.dt.bfloat16
    nc = bacc.Bacc("TRN2", debug=False, num_devices=NCORES)

    mt = nc.dram_tensor("mt", [E, VS], bf16, kind="ExternalInput").ap()
    de = nc.dram_tensor("de", [B, E, L], bf16, kind="ExternalInput").ap()
    # f32 copies for the numerator path: the final loss is dominated by the
    # largest few exp(s/tau) tokens, so s must be computed at f32 precision.
    def_ = nc.dram_tensor("def", [B, E, L], f32, kind="ExternalInput").ap()
    enf = nc.dram_tensor("enf", [B, E, L], f32, kind="ExternalInput").ap()
    # down_acc[p, tb*4+h] = sum over one 1000-col vocab slice of exp(dist/tau)
    # for token (b=tb//4, l=(tb%4)*128+p)
    down_acc = nc.dram_tensor("down_acc", [128, NTB * 4], f32, kind="ExternalOutput").ap()
    # s_out[b, l] = sum_e EN[b,e,l]*DE[b,e,l]  (pre-exp numerator dots)
    s_out = nc.dram_tensor("s_out", [B, L], f32, kind="ExternalOutput").ap()

    with tile.TileContext(nc) as tc, ExitStack() as ctx:
        mt_pool = ctx.enter_context(tc.tile_pool(name="mtp", bufs=1))
        de_pool = ctx.enter_context(tc.tile_pool(name="dep", bufs=1))
        en_pool = ctx.enter_context(tc.tile_pool(name="enp", bufs=2))
        tmp_pool = ctx.enter_context(tc.tile_pool(name="tmpp", bufs=2))
        s_pool = ctx.enter_context(tc.tile_pool(name="sp", bufs=2))
        acc_pool = ctx.enter_context(tc.tile_pool(name="accp", bufs=1))
        ps_pool = ctx.enter_context(tc.tile_pool(name="psp", bufs=3, space="PSUM"))
        ups_pool = ctx.enter_context(tc.tile_pool(name="upsp", bufs=2, space="PSUM"))

        ones = acc_pool.tile([128, 1], bf16, tag="ones", name="ones")
        nc.vector.memset(ones[:], 1.0)
        acc = acc_pool.tile([128, NTB * 4], f32, tag="acc", name="acc")

        # DRAM views folding the e dim as (k p): partition p, e-block k.
        de_r = de.rearrange("b (k p) l -> b p k l", p=128)
        mt_r = mt.rearrange("(k p) v -> p k v", p=128)
        enf_r = enf.rearrange("b (k p) l -> b p k l", p=128)
        def_r = def_.rearrange("b (k p) l -> b p k l", p=128)

        # Stationary weights: one [p128, k4, l512] tile per batch entry, one
        # DMA each.  Moving operand: transposed-M tiles [p128, k4, v500],
        # one DMA per vocab block.  Issue order matches first use under the
        # v-outer main loop: de0, mt_v0, de1..3, mt_v1..7.
        de_sb = {}
        mt_sb = {}
        # First-needed tiles split by e-block so the very first matmul only
        # waits on a 64KB chunk (subtile deps track the per-k writes).
        t = de_pool.tile([128, NKB, L], bf16, tag="de0", name="de0")
        t2 = mt_pool.tile([128, NKB, VBLK], bf16, tag="mt0", name="mt0")
        t3 = mt_pool.tile([128, NKB, VBLK], bf16, tag="mt1", name="mt1")
        for k in range(NKB):
            nc.sync.dma_start(out=t[:, k, :], in_=de_r[0][:, k, :])
            nc.sync.dma_start(out=t2[:, k, :], in_=mt_r[:, k, 0:VBLK])
        de_sb[0] = t
        mt_sb[0] = t2
        for k in range(NKB):
            nc.sync.dma_start(out=t3[:, k, :], in_=mt_r[:, k, VBLK : 2 * VBLK])
        mt_sb[1] = t3
        for b in range(1, B):
            t = de_pool.tile([128, NKB, L], bf16, tag=f"de{b}", name=f"de{b}")
            nc.sync.dma_start(out=t[:], in_=de_r[b])
            de_sb[b] = t
        for v in range(2, NVB):
            t2 = mt_pool.tile([128, NKB, VBLK], bf16, tag=f"mt{v}", name=f"mt{v}")
            if v < 4:
                half = NKB // 2
                src = mt_r[:, :, v * VBLK : (v + 1) * VBLK]
                nc.sync.dma_start(out=t2[:, :half, :], in_=src[:, :half, :])
                nc.sync.dma_start(out=t2[:, half:, :], in_=src[:, half:, :])
            else:
                nc.sync.dma_start(out=t2[:], in_=mt_r[:, :, v * VBLK : (v + 1) * VBLK])
            mt_sb[v] = t2

        # Dummy matmuls on a zeroed tile: keep the PE busy while the first
        # real operands stream in, so the HAM clock-gate is already warm
        # (2.4GHz) when the real matmuls start.
        warm = acc_pool.tile([128, 128], bf16, tag="warm", name="warm")
        nc.vector.memset(warm[:], 0.0)
        wps = ps_pool.tile([128, 2, 512], f32, tag="ps", name="warmps")
        for i in range(32):
            nc.tensor.matmul(
                wps[:, 0, 0:128], lhsT=warm[:], rhs=warm[:], start=True, stop=True
            )

        # Numerator path (all f32): tm_k = EN*DE per e-block on DVE, partial
        # partition-group sums folded elementwise into per-b tsum tiles.
        # All DVE work runs during the main loop; the four ones-matmuls that
        # reduce the remaining 128 partitions are spliced into the middle of
        # the PE stream (below) once the tsums are long since ready.
        tsum_sb = {}
        for b in range(B):
            et = en_pool.tile([128, NKB, L], f32, tag="en", name=f"en{b}")
            nc.sync.dma_start(out=et[:], in_=enf_r[b])
            dt = en_pool.tile([128, NKB, L], f32, tag="def", name=f"def{b}")
            nc.sync.dma_start(out=dt[:], in_=def_r[b])
            tsum = tmp_pool.tile([128, L], f32, tag=f"tsum{b}", name=f"tsum{b}")
            tsum_sb[b] = tsum
            for k in range(NKB):
                if k == 0:
                    nc.vector.tensor_mul(tsum[:], et[:, 0, :], dt[:, 0, :])
                else:
                    tm = tmp_pool.tile([128, L], f32, tag="tmp", name=f"tm{b}_{k}")
                    nc.vector.tensor_mul(tm[:], et[:, k, :], dt[:, k, :])
                    nc.vector.tensor_add(tsum[:], tsum[:], tm[:])
            # hi/lo bf16 split of tsum: the ones-matmul then runs as two
            # full-rate bf16 passes instead of an f32 LOW_HIGH pair, with
            # the same f32 accuracy (hi + lo reconstructs tsum exactly to
            # ~2^-17 relative; PSUM accumulates in f32).
            hib = tmp_pool.tile([128, L], bf16, tag=f"hi{b}", name=f"hi{b}")
            nc.vector.tensor_copy(hib[:], tsum[:])
            dif = tmp_pool.tile([128, L], f32, tag="dif", name=f"dif{b}")
            nc.vector.tensor_sub(dif[:], tsum[:], hib[:])
            lob = tmp_pool.tile([128, L], bf16, tag=f"lo{b}", name=f"lo{b}")
            nc.vector.tensor_copy(lob[:], dif[:])
            tsum_sb[b] = (hib, lob)

        # Main loop, vocab-block-pair outer / token-block inner: the first
        # 128 matmuls touch only mt_v0/v1, so later mt DMAs hide behind
        # compute.  Per (h, tb): accumulate over e into a 2-bank PSUM tile,
        # then one fused exp+sum on ACT.
        for h in range(NVB // 2):
            for tb in range(NTB):
                b, lb = divmod(tb, NLB)
                ps = ps_pool.tile([128, 2, 512], f32, tag="ps", name=f"ps{tb}_{h}")
                for j in range(2):
                    v = h * 2 + j
                    for k in range(NKB):
                        nc.tensor.matmul(
                            ps[:, j, 0:VBLK],
                            lhsT=de_sb[b][:, k, lb * 128 : (lb + 1) * 128],
                            rhs=mt_sb[v][:, k, :],
                            start=(k == 0),
                            stop=(k == NKB - 1),
                        )
                nc.scalar.activation(
                    out=ps[:, :, 0:VBLK],
                    in_=ps[:, :, 0:VBLK],
                    func=mybir.ActivationFunctionType.Exp,
                    scale=INV_TAU,
                    accum_out=acc[:, h * NTB + tb : h * NTB + tb + 1],
                )
            if h == 1:
                # Ones-matmul partition reductions of the numerator, spliced
                # mid-stream so they don't extend the PE tail.
                for b in range(B):
                    hib, lob = tsum_sb[b]
                    ups = ups_pool.tile([1, L], f32, tag="ups", name=f"ups{b}")
                    nc.tensor.matmul(
                        ups[:], lhsT=ones[:], rhs=hib[:], start=True, stop=False
                    )
                    nc.tensor.matmul(
                        ups[:], lhsT=ones[:], rhs=lob[:], start=False, stop=True
                    )
                    ssb = s_pool.tile([1, L], f32, tag="ssb", name=f"ssb{b}")
                    nc.vector.tensor_copy(ssb[:], ups[:])
                    nc.sync.dma_start(out=s_out[b : b + 1, :], in_=ssb[:])
                # First half of the partial-denominator output (h=0,1) can
                # ship while the second half computes.
                nc.sync.dma_start(
                    out=down_acc[:, 0 : 2 * NTB], in_=acc[:, 0 : 2 * NTB]
                )

        nc.sync.dma_start(out=down_acc[:, 2 * NTB :], in_=acc[:, 2 * NTB :])

    nc.compile()
    return nc


def kernel(embed_EN, embed_DE, embed_M):
    global LAST_RESULTS
    import ml_dtypes

    from concourse.bass_utils import run_bass_kernel_spmd

    if "nc" not in _CACHE:
        _CACHE["nc"] = _build()
    nc = _CACHE["nc"]

    bf16 = ml_dtypes.bfloat16
    enf = np.ascontiguousarray(np.asarray(embed_EN, dtype=np.float32))
    def_ = np.ascontiguousarray(np.asarray(embed_DE, dtype=np.float32))
    de = np.ascontiguousarray(def_.astype(bf16))
    mt_full = np.ascontiguousarray(
        np.asarray(embed_M, dtype=np.float32).T.astype(bf16)
    )  # [E, V]

    in_maps = [
        {
            "mt": np.ascontiguousarray(mt_full[:, c * VS : (c + 1) * VS]),
            "de": de,
            "def": def_,
            "enf": enf,
        }
        for c in range(NCORES)
    ]

    res = run_bass_kernel_spmd(nc, in_maps, core_ids=list(range(NCORES)))
    LAST_RESULTS = res

    # Gather: all-reduce the partial denominators across cores, then the
    # final division + scalar sum (done in f64 for a clean f32 result).
    acc_sum = np.zeros((128, NTB * 4), np.float64)
    for r in res.results:
        acc_sum += r["down_acc"].astype(np.float64)
    down = acc_sum.reshape(128, 4, NTB).sum(1)           # [p, tb]
    down = down.T.reshape(B, NLB, 128).reshape(B, L)     # [b, l=lb*128+p]
    s = res.results[0]["s_out"].astype(np.float64)       # [b, l]
    up = np.exp(INV_TAU * s)
    return np.asarray((up / down).sum(), dtype=np.float32)
